# revision 1
# baseline (speedup 1.0000x reference)
"""MoE-routed multi-head attention kernel for 8 Trainium2 NeuronCores.

Problem shape (hardcoded):
  query/key/value: [4, 2048, 512] f32
  Wg [512,8], Wk/Wv [512,64], Wq [8,512,64], Wo [8,64,512], biases.
  TOP_K=2 routed experts act as the two attention heads.

Sharding: core c = 2*b + h handles batch b, query-half h (1024 query tokens),
with the full 2048 keys/values of batch b. All compute stays on device; the
host only slices/transposes/casts inputs and concatenates outputs.

Numerics: matmul operands are bf16 (fp32 PSUM accumulation); the router is
computed exactly via a 3-term bf16 hi/lo split so top-2 expert selection
matches fp32; softmax normalization, gates and combine scalars are fp32.

Structure: per-phase emission is engine-contiguous (all PE work of a stage,
then all DVE work, ...) so no engine blocks inside another engine's serial
chain, and inputs arrive in 6 large packed DMAs.
"""

import numpy as np

import concourse.bass as bass
import concourse.mybir as mybir
import concourse.tile as tile
from concourse import bacc
from concourse import bass_utils
from concourse.masks import make_identity

P = 128
D = 512          # d_model
T = 2048         # kv tokens per core (full batch)
NQ = 1024        # query tokens per core
E = 8            # experts
DK = 64          # head dim
DC = D // P      # 4 contraction chunks
NKC = T // P     # 16 key chunks
NQT = NQ // P    # 8 query tiles
VW = DK + 1      # vh columns + ones column (denominator trick)
HD = D           # phase-C column granularity (half of NQ)

FP = mybir.dt.float32
U32 = mybir.dt.uint32
BF = mybir.dt.bfloat16
AF = mybir.ActivationFunctionType
OP = mybir.AluOpType
AX = mybir.AxisListType

DEBUG = False

# packed weight layout (columns in wpack, all bf16):
#   per dc (4x): Wk2 128 | Wv 64 | Wq_f 512 | Wo_f 512 | Wg_hi 8 | Wg_lo 8
_WCOLS = [("Wk2", P), ("Wv", DK), ("Wq_f", DK * E), ("Wo_f", D),
          ("Wg_hi", E), ("Wg_lo", E)]
_WSTRIDE = sum(c for _, c in _WCOLS)          # 1232
# extras appended after the 4 dc blocks:
#   bq_f 512 (row 0) | bo 512 (rows 0-7) | ones1 128 (row 0) | onescol 16
_XBASE = 4 * _WSTRIDE
_WPACK_COLS = _XBASE + D + D + P + NKC


def _emit(nc, tc, ctx):
    const = ctx.enter_context(tc.tile_pool(name="const", bufs=1))
    persist = ctx.enter_context(tc.tile_pool(name="persist", bufs=1))
    work = ctx.enter_context(tc.tile_pool(name="work", bufs=3))
    expp = ctx.enter_context(tc.tile_pool(name="expp", bufs=4))
    ps_log = ctx.enter_context(tc.tile_pool(name="ps_log", bufs=4, space="PSUM"))
    ps_att = ctx.enter_context(tc.tile_pool(name="ps_att", bufs=2, space="PSUM"))
    ps_sm = ctx.enter_context(tc.tile_pool(name="ps_sm", bufs=2, space="PSUM"))

    dram = {}
    for name, shape, dt in [
        ("qhi_pack", [P, DC * NQ], BF), ("qlo_pack", [P, DC * NQ], BF),
        ("kT_pack", [P, DC * T], BF), ("vT_pack", [P, DC * T], BF),
        ("wpack", [P, _WPACK_COLS], BF), ("fpack", [P, 2], FP),
    ]:
        dram[name] = nc.dram_tensor(name, shape, dt, kind="ExternalInput").ap()
    out_d = nc.dram_tensor("out", [NQ, D], FP, kind="ExternalOutput").ap()
    if DEBUG:
        dbg_lg = nc.dram_tensor("dbg_lg", [NQ, E], FP, kind="ExternalOutput").ap()
        dbg_i = nc.dram_tensor("dbg_i", [NQ, E], U32, kind="ExternalOutput").ap()
        dbg_g = nc.dram_tensor("dbg_g", [NQ, 2], FP, kind="ExternalOutput").ap()

    # ---- bulk input loads (6 DMAs) ----
    wpack = const.tile([P, _WPACK_COLS], BF, tag="wpack")
    nc.sync.dma_start(wpack[:], dram["wpack"])
    fpack = const.tile([P, 2], FP, tag="fpack")
    nc.sync.dma_start(fpack[:], dram["fpack"])
    qhi = persist.tile([P, DC * NQ], BF, tag="qhi")
    nc.sync.dma_start(qhi[:], dram["qhi_pack"])
    qlo = persist.tile([P, DC * NQ], BF, tag="qlo")
    nc.sync.dma_start(qlo[:], dram["qlo_pack"])
    kTt = persist.tile([P, DC * T], BF, tag="kTt")
    nc.sync.dma_start(kTt[:], dram["kT_pack"])
    vTt = persist.tile([P, DC * T], BF, tag="vTt")
    nc.sync.dma_start(vTt[:], dram["vT_pack"])

    w = {}
    for dc in range(DC):
        off = dc * _WSTRIDE
        for name, cols in _WCOLS:
            w[(name, dc)] = wpack[:, off:off + cols]
            off += cols
    bq_f = wpack[0:1, _XBASE:_XBASE + D]
    bo = wpack[0:E, _XBASE + D:_XBASE + 2 * D]
    ones_row = wpack[0:1, _XBASE + 2 * D:_XBASE + 2 * D + P]
    onescol = wpack[:, _XBASE + 2 * D + P:_XBASE + 2 * D + P + NKC]
    bkT2 = fpack[:, 0:1]
    bvT = fpack[0:DK, 1:2]
    qT_hi = {dc: qhi[:, dc * NQ:(dc + 1) * NQ] for dc in range(DC)}
    qT_lo = {dc: qlo[:, dc * NQ:(dc + 1) * NQ] for dc in range(DC)}
    kT = {dc: kTt[:, dc * T:(dc + 1) * T] for dc in range(DC)}
    vT = {dc: vTt[:, dc * T:(dc + 1) * T] for dc in range(DC)}

    # ---- constants ----
    ident = const.tile([P, P], FP, tag="ident")
    make_identity(nc, ident[:])
    ident_b = const.tile([P, P], BF, tag="ident_b")
    make_identity(nc, ident_b[:])
    iota_e = const.tile([P, DK * E], BF, tag="iota_e")  # expert idx, (d e) cols
    nc.gpsimd.iota(iota_e[:].rearrange("p (d e) -> p d e", e=E),
                   pattern=[[0, DK], [1, E]], channel_multiplier=0,
                   allow_small_or_imprecise_dtypes=True)
    iota8 = const.tile([P, E], FP, tag="iota8")
    nc.gpsimd.iota(iota8[:], pattern=[[1, E]], channel_multiplier=0,
                   allow_small_or_imprecise_dtypes=True)

    # ---- Phase A: khT2 [128, T] (head-stacked), vh_aug; router logits ----
    khT2 = persist.tile([P, T], BF, tag="khT2")
    vhT = persist.tile([DK, T], BF, tag="vhT")
    for ncH in range(4):
        cs = slice(ncH * HD, (ncH + 1) * HD)
        ps = ps_sm.tile([P, HD], FP, tag="ps")
        for dc in range(DC):
            nc.tensor.matmul(ps[:], w[("Wk2", dc)][:], kT[dc][:, cs],
                             start=(dc == 0), stop=(dc == DC - 1))
        nc.vector.tensor_scalar(khT2[:, cs], ps[:], bkT2, None, op0=OP.add)
    for ncH in range(4):
        cs = slice(ncH * HD, (ncH + 1) * HD)
        ps = ps_sm.tile([DK, HD], FP, tag="ps")
        for dc in range(DC):
            nc.tensor.matmul(ps[:], w[("Wv", dc)][:], vT[dc][:, cs],
                             start=(dc == 0), stop=(dc == DC - 1))
        nc.vector.tensor_scalar(vhT[:, cs], ps[:], bvT, None, op0=OP.add)

    routerT = persist.tile([E, NQ], FP, tag="routerT")
    for half in range(2):
        hs = slice(half * HD, (half + 1) * HD)
        ps_rt = ps_sm.tile([E, HD], FP, tag="ps")
        first = True
        for dc in range(DC):
            for lname, r in [("Wg_hi", qT_hi), ("Wg_hi", qT_lo),
                             ("Wg_lo", qT_hi)]:
                nc.tensor.matmul(ps_rt[:], w[(lname, dc)][:], r[dc][:, hs],
                                 start=first,
                                 stop=(dc == DC - 1 and lname == "Wg_lo"))
                first = False
        nc.vector.tensor_copy(routerT[:, hs], ps_rt[:])

    vh_aug = persist.tile([P, NKC * VW], BF, tag="vh_aug")
    nc.vector.tensor_copy(
        vh_aug[:].rearrange("p (c w) -> p c w", w=VW)[:, :, DK], onescol)
    for kc in range(NKC):
        ps = ps_sm.tile([P, P], BF, tag="ps")
        nc.tensor.matmul(ps[:, :DK], vhT[:, kc * P:(kc + 1) * P],
                         ident_b[:DK, :DK], is_transpose=True)
        nc.vector.tensor_copy(vh_aug[:, kc * VW:kc * VW + DK], ps[:, :DK])

    # ---- Phase B1: all-expert query projections + router logit tiles (PE) ----
    qa_bs, lg8s, m8s, if8s, mreps = {}, {}, {}, {}, {}
    for qt in range(NQT):
        qs = slice(qt * P, (qt + 1) * P)
        ps_qa = ps_sm.tile([P, DK * E], FP, tag="ps")
        for dc in range(DC):
            nc.tensor.matmul(ps_qa[:], qT_hi[dc][:, qs], w[("Wq_f", dc)][:],
                             start=(dc == 0), stop=False)
        nc.tensor.matmul(ps_qa[:], ones_row, bq_f, start=False, stop=True)
        qa_b = persist.tile([P, DK * E], BF, tag=f"qa_b{qt}", name=f"qa_b{qt}")
        nc.scalar.activation(qa_b[:], ps_qa[:], AF.Copy)
        qa_bs[qt] = qa_b
        ps_r = ps_sm.tile([P, E], FP, tag="ps")
        nc.tensor.matmul(ps_r[:], routerT[:, qs], ident[:E, :E], is_transpose=True)
        lg8 = persist.tile([P, E], FP, tag=f"lg8_{qt}", name=f"lg8_{qt}")
        nc.vector.tensor_copy(lg8[:], ps_r[:])
        lg8s[qt] = lg8

    # ---- Phase B2: top-2 select + masked query gather (DVE) + transposes ----
    qselT2 = persist.tile([P, NQ], BF, tag="qselT2")  # h0 rows 0-63, h1 64-127
    for qt in range(NQT):
        qs = slice(qt * P, (qt + 1) * P)
        m8 = persist.tile([P, E], FP, tag=f"m8_{qt}", name=f"m8_{qt}")
        nc.vector.max(out=m8[:], in_=lg8s[qt][:])
        m8s[qt] = m8
        i8 = work.tile([P, E], U32, tag="i8")
        nc.vector.max_index(i8[:], m8[:], lg8s[qt][:])
        if8 = persist.tile([P, 2], FP, tag=f"if8_{qt}", name=f"if8_{qt}")
        nc.vector.tensor_copy(if8[:], i8[:, 0:2])
        if8s[qt] = if8
        if DEBUG:
            nc.sync.dma_start(dbg_lg[qs, :], lg8s[qt][:])
            nc.sync.dma_start(dbg_i[qs, :], i8[:])
        qsel2 = work.tile([P, P], FP, tag="qsel2")
        for h in range(2):
            mrep = persist.tile([P, DK * E], BF, tag=f"mrep{qt}_{h}",
                                name=f"mrep{qt}_{h}")
            nc.vector.tensor_scalar(mrep[:], iota_e[:], if8[:, h:h + 1], None,
                                    op0=OP.is_equal)
            mreps[(qt, h)] = mrep
            u = work.tile([P, DK * E], BF, tag=f"u{h}", name=f"u{h}")
            nc.vector.tensor_tensor(u[:], qa_bs[qt][:], mrep[:], op=OP.mult)
            nc.vector.reduce_sum(qsel2[:, h * DK:(h + 1) * DK],
                                 u[:].rearrange("p (d e) -> p d e", e=E),
                                 axis=AX.X)
        ps_t = ps_sm.tile([P, P], FP, tag="ps")
        nc.tensor.matmul(ps_t[:], qsel2[:], ident[:], is_transpose=True)
        nc.vector.tensor_copy(qselT2[:, qs], ps_t[:])

    # ---- Phase B3: softmax gates + combine weights (needed only in D) ----
    combT = persist.tile([E, NQ], BF, tag="combT")
    g_sb = [persist.tile([P, NQT], FP, tag=f"g{h}", name=f"g{h}") for h in range(2)]
    for qt in range(NQT):
        qs = slice(qt * P, (qt + 1) * P)
        e8 = work.tile([P, E], FP, tag="e8")
        nc.scalar.activation(e8[:], lg8s[qt][:], AF.Exp)
        gtop = work.tile([P, 2], FP, tag="gtop")
        nc.scalar.activation(gtop[:], m8s[qt][:, 0:2], AF.Exp)
        ssum = work.tile([P, 1], FP, tag="ssum")
        nc.vector.reduce_sum(ssum[:], e8[:], axis=AX.X)
        srec = work.tile([P, 1], FP, tag="srec")
        nc.vector.reciprocal(srec[:], ssum[:])
        for h in range(2):
            nc.vector.tensor_tensor(g_sb[h][:, qt:qt + 1], gtop[:, h:h + 1],
                                    srec[:], op=OP.mult)
        if DEBUG:
            for h in range(2):
                nc.sync.dma_start(dbg_g[qs, h:h + 1], g_sb[h][:, qt:qt + 1])
        comb8 = work.tile([P, E], FP, tag="comb8")
        tmp8 = work.tile([P, E], FP, tag="tmp8")
        nc.vector.scalar_tensor_tensor(
            comb8[:], iota8[:], if8s[qt][:, 0:1],
            g_sb[0][:, qt:qt + 1].to_broadcast((P, E)), op0=OP.is_equal, op1=OP.mult)
        nc.vector.scalar_tensor_tensor(
            tmp8[:], iota8[:], if8s[qt][:, 1:2],
            g_sb[1][:, qt:qt + 1].to_broadcast((P, E)), op0=OP.is_equal, op1=OP.mult)
        nc.vector.tensor_tensor(comb8[:], comb8[:], tmp8[:], op=OP.add)
        ps_c = ps_sm.tile([E, P], FP, tag="ps")
        nc.tensor.matmul(ps_c[:], comb8[:], ident[:], is_transpose=True)
        nc.vector.tensor_copy(combT[:, qs], ps_c[:])

    # ---- Phase C: attention, halves outer, heads packed in PE row groups ----
    attnT = [persist.tile([VW, NQ], FP, tag=f"attnT{h}", name=f"attnT{h}")
             for h in range(2)]
    for half in range(2):
        hs = slice(half * HD, (half + 1) * HD)
        ps_a = [ps_att.tile([VW, HD], FP, tag="ps_a", name=f"ps_a{h}")
                for h in range(2)]
        pending = None  # software pipeline: attn MMs trail logits by one chunk
        for kc in range(NKC):
            ps_l = [ps_log.tile([P, HD], FP, tag="ps_l", name=f"ps_l{h}")
                    for h in range(2)]
            for h in range(2):
                rg = slice(h * DK, (h + 1) * DK)
                nc.tensor.matmul(ps_l[h][:], khT2[rg, kc * P:(kc + 1) * P],
                                 qselT2[rg, hs], start=True, stop=True)
            if pending is not None:
                pkc, pex = pending
                for h in range(2):
                    nc.tensor.matmul(ps_a[h][:], vh_aug[:, pkc * VW:(pkc + 1) * VW],
                                     pex[h][:], start=(pkc == 0),
                                     stop=(pkc == NKC - 1), skip_group_check=True)
            ex = [expp.tile([P, HD], BF, tag="ex", name=f"ex{h}") for h in range(2)]
            for h in range(2):
                nc.scalar.activation(ex[h][:], ps_l[h][:], AF.Exp, scale=0.125)
            pending = (kc, ex)
        pkc, pex = pending
        for h in range(2):
            nc.tensor.matmul(ps_a[h][:], vh_aug[:, pkc * VW:(pkc + 1) * VW],
                             pex[h][:], start=(pkc == 0), stop=(pkc == NKC - 1),
                             skip_group_check=True)
        for h in range(2):
            nc.vector.tensor_copy(attnT[h][:, hs], ps_a[h][:])

    # ---- Phase D1: attention transposes back to token-major (PE + copies) ----
    ats = {}
    for qt in range(NQT):
        qs = slice(qt * P, (qt + 1) * P)
        for h in range(2):
            ps_t = ps_sm.tile([P, VW], FP, tag="ps")
            nc.tensor.matmul(ps_t[:], attnT[h][:, qs], ident[:VW, :VW],
                             is_transpose=True)
            a = persist.tile([P, VW], FP, tag=f"at{qt}_{h}", name=f"at{qt}_{h}")
            nc.vector.tensor_copy(a[:], ps_t[:])
            ats[(qt, h)] = a

    # ---- Phase D2: per-token combine scalars + masked scatter (DVE) ----
    cms = {}
    for qt in range(NQT):
        ch = []
        for h in range(2):
            a = ats[(qt, h)]
            dinv = work.tile([P, 1], FP, tag="dinv")
            nc.vector.reciprocal(dinv[:], a[:, DK:DK + 1])
            s = work.tile([P, 1], FP, tag="s")
            nc.vector.tensor_tensor(s[:], g_sb[h][:, qt:qt + 1], dinv[:], op=OP.mult)
            c = work.tile([P, DK * E], BF, tag=f"c{h}", name=f"c{h}")
            nc.vector.scalar_tensor_tensor(
                c[:].rearrange("p (d e) -> p d e", e=E),
                a[:, :DK].unsqueeze(2).broadcast_to((P, DK, E)),
                s[:],
                mreps[(qt, h)][:].rearrange("p (d e) -> p d e", e=E),
                op0=OP.mult, op1=OP.mult)
            ch.append(c)
        cm = persist.tile([P, DK * E], BF, tag=f"cm{qt}", name=f"cm{qt}")
        nc.vector.tensor_tensor(cm[:], ch[0][:], ch[1][:], op=OP.add)
        cms[qt] = cm

    # ---- Phase D3: transpose c, output projection, store ----
    for qt in range(NQT):
        qs = slice(qt * P, (qt + 1) * P)
        cTs = []
        for ci in range(DC):
            ps_ct = ps_sm.tile([P, P], BF, tag="ps")
            nc.tensor.matmul(ps_ct[:], cms[qt][:, ci * P:(ci + 1) * P], ident_b[:],
                             is_transpose=True)
            cT = work.tile([P, P], BF, tag=f"cT{ci}", name=f"cT{ci}")
            nc.vector.tensor_copy(cT[:], ps_ct[:])
            cTs.append(cT)
        ps_o = ps_sm.tile([P, D], FP, tag="ps")
        for ci in range(DC):
            nc.tensor.matmul(ps_o[:], cTs[ci][:], w[("Wo_f", ci)][:],
                             start=(ci == 0), stop=False)
        nc.tensor.matmul(ps_o[:], combT[:, qs], bo, start=False, stop=True)
        o = work.tile([P, D], FP, tag="o")
        nc.scalar.activation(o[:], ps_o[:], AF.Copy)
        nc.sync.dma_start(out_d[qs, :], o[:])


_PROGRAM = None


def get_program():
    global _PROGRAM
    if _PROGRAM is None:
        nc = bacc.Bacc("TRN2", target_bir_lowering=False, debug=False,
                       enable_asserts=False, num_devices=8)
        from contextlib import ExitStack
        with tile.TileContext(nc) as tc, ExitStack() as ctx:
            _emit(nc, tc, ctx)
        nc.compile()
        _PROGRAM = nc
    return _PROGRAM


def make_in_maps(query, key, value, Wg, Wk, bk, Wv, bv, Wq, bq, Wo, bo):
    import ml_dtypes
    BFNP = ml_dtypes.bfloat16

    def hilo(x):
        x = np.asarray(x, np.float32)
        hi = x.astype(BFNP)
        lo = (x - hi.astype(np.float32)).astype(BFNP)
        return hi, lo

    Wg_hi, Wg_lo = hilo(Wg)
    Wk2 = np.concatenate([np.asarray(Wk), np.asarray(Wk)], axis=1)  # [512, 128]
    Wq_f = np.asarray(Wq).transpose(1, 2, 0).reshape(D, DK * E)
    Wo_f = np.asarray(Wo).transpose(1, 0, 2).reshape(DK * E, D)
    wparts = {"Wk2": np.asarray(Wk2, BFNP), "Wv": np.asarray(Wv, BFNP),
              "Wq_f": np.asarray(Wq_f, BFNP), "Wo_f": np.asarray(Wo_f, BFNP),
              "Wg_hi": Wg_hi, "Wg_lo": Wg_lo}
    wpack = np.zeros((P, _WPACK_COLS), BFNP)
    for dc in range(DC):
        off = dc * _WSTRIDE
        rows = slice(dc * P, (dc + 1) * P)
        for name, cols in _WCOLS:
            wpack[:, off:off + cols] = wparts[name][rows, :]
            off += cols
    wpack[0, _XBASE:_XBASE + D] = \
        np.asarray(bq, np.float32).T.reshape(DK * E).astype(BFNP)
    wpack[0:E, _XBASE + D:_XBASE + 2 * D] = np.asarray(bo, BFNP)
    wpack[0, _XBASE + 2 * D:_XBASE + 2 * D + P] = np.ones(P, BFNP)
    wpack[:, _XBASE + 2 * D + P:_XBASE + 2 * D + P + NKC] = np.ones((P, NKC), BFNP)
    fpack = np.zeros((P, 2), np.float32)
    fpack[:, 0] = np.concatenate([np.asarray(bk), np.asarray(bk)])
    fpack[0:DK, 1] = np.asarray(bv)

    def pack_chunks(x):  # [512, N] -> [128, 4*N] (dc-major columns)
        n = x.shape[1]
        out = np.empty((P, DC * n), x.dtype)
        for dc in range(DC):
            out[:, dc * n:(dc + 1) * n] = x[dc * P:(dc + 1) * P, :]
        return np.ascontiguousarray(out)

    shared = {"wpack": np.ascontiguousarray(wpack), "fpack": fpack}
    in_maps = []
    for b in range(4):
        kTp = pack_chunks(np.asarray(key[b], np.float32).T.astype(BFNP))
        vTp = pack_chunks(np.asarray(value[b], np.float32).T.astype(BFNP))
        for h in range(2):
            qhi, qlo = hilo(np.asarray(query[b][h * NQ:(h + 1) * NQ, :]).T)
            in_maps.append({"kT_pack": kTp, "vT_pack": vTp,
                            "qhi_pack": pack_chunks(qhi),
                            "qlo_pack": pack_chunks(qlo), **shared})
    return in_maps


def kernel(query, key, value, Wg, Wk, bk, Wv, bv, Wq, bq, Wo, bo):
    in_maps = make_in_maps(query, key, value, Wg, Wk, bk, Wv, bv, Wq, bq, Wo, bo)
    nc = get_program()
    res = bass_utils.run_bass_kernel_spmd(nc, in_maps, core_ids=list(range(8)))
    outs = [res.results[c]["out"] for c in range(8)]
    return np.concatenate(outs, axis=0).reshape(4, T, D).astype(np.float32)



# revision 5
# speedup vs baseline: 1.3435x; 1.3435x over previous
"""MoE-routed multi-head attention kernel for 8 Trainium2 NeuronCores.

Problem shape (hardcoded):
  query/key/value: [4, 2048, 512] f32
  Wg [512,8], Wk/Wv [512,64], Wq [8,512,64], Wo [8,64,512], biases.
  TOP_K=2 routed experts act as the two attention heads.

Sharding: core c = 2*b + h handles batch b, query-half h (1024 query tokens),
with the full 2048 keys/values of batch b.

Key structural choices (v2):
  - bk is dropped: adding bk to kh shifts every logit of a query by a
    constant, which softmax ignores.  bv is folded into bo on the host
    (bo' = bo + bv @ Wo_e) since attn rows sum to 1 before the gate.
  - vh is computed directly key-major (vT chunks stationary, Wv moving),
    so no PE transposes of vh are needed.
  - Router logits (3-term bf16 hi/lo, fp32-exact for top-2 selection) are
    computed T-layout with tiny Wg-stationary loads, transposed per qt.
  - The expert gather (qsel from q_all) is a chain of 8 fused
    scalar_tensor_tensor ops per head; the expert scatter (attn -> cm)
    is a gpsimd local_scatter with per-token int16 indices.
  - Phase C: per kc one [128,1024] fp32 2-bank PSUM logits tile (both
    heads via row-grouped concurrent matmuls), one [128,1024] exp, and
    2 attention matmuls, software-pipelined.  B-blocks for qts 4-7 are
    interleaved into C(half0)'s PE stream, D-blocks into C(half1)'s, so
    no engine idles and the PE never sees a matmul-free window (HAM).
  - Output is written bf16; host casts to fp32.
"""

import numpy as np

import concourse.bass as bass
import concourse.mybir as mybir
import concourse.tile as tile
from concourse import bacc
from concourse import bass_utils
from concourse.masks import make_identity

P = 128
D = 512          # d_model
T = 2048         # kv tokens per core (full batch)
NQ = 1024        # query tokens per core
E = 8            # experts
DK = 64          # head dim
DC = D // P      # 4 contraction chunks
NKC = T // P     # 16 key chunks
NQT = NQ // P    # 8 query tiles
VW = DK + 1      # vh columns + ones column (denominator trick)
HD = 512         # phase-C column granularity (half of NQ)

FP = mybir.dt.float32
U32 = mybir.dt.uint32
I16 = mybir.dt.int16
BF = mybir.dt.bfloat16
AF = mybir.ActivationFunctionType
OP = mybir.AluOpType
AX = mybir.AxisListType

USE_GPSIMD_SCATTER = True
ACT_ACCUM = True         # activation accum_out for softmax row sums

# ---- w0a packed layout (bf16 columns) ----
_W0A = {}
_off = 0
for name, cols in [("Wk2s", DC * P), ("Wv", DC * DK), ("Wg_hi", DC * E),
                   ("Wg_lo", DC * E), ("ones", P), ("bq_row", D), ("bo", D)]:
    _W0A[name] = _off
    _off += cols
_W0A_COLS = _off
_W0B_COLS = 2 * DC * D   # Wq_f (e d) | Wo_f (e d)


def _emit(nc, tc, ctx):
    const = ctx.enter_context(tc.tile_pool(name="const", bufs=1))
    persist = ctx.enter_context(tc.tile_pool(name="persist", bufs=1))
    work = ctx.enter_context(tc.tile_pool(name="work", bufs=3))
    expp = ctx.enter_context(tc.tile_pool(name="expp", bufs=3))
    dpool = ctx.enter_context(tc.tile_pool(name="dpool", bufs=8))
    ps_log = ctx.enter_context(tc.tile_pool(name="ps_log", bufs=2, space="PSUM"))
    ps_att = ctx.enter_context(tc.tile_pool(name="ps_att", bufs=2, space="PSUM"))
    ps_m = ctx.enter_context(tc.tile_pool(name="ps_m", bufs=2, space="PSUM"))

    dram = {}
    for name, shape, dt in [
        ("w0a", [P, _W0A_COLS], BF), ("w0b", [P, _W0B_COLS], BF),
        ("kTn", [P, DC * T], BF), ("vTn", [P, DC * T], BF),
        ("qhi", [P, DC * NQ], BF), ("qlo", [P, DC * NQ], BF),
    ]:
        dram[name] = nc.dram_tensor(name, shape, dt, kind="ExternalInput").ap()
    out_d = nc.dram_tensor("out", [NQ, D], BF, kind="ExternalOutput").ap()

    # ---- DMAs, ordered by first use; kT split so khT2 starts early ----
    w0a = const.tile([P, _W0A_COLS], BF, tag="w0a")
    nc.sync.dma_start(w0a[:], dram["w0a"])
    kTt = persist.tile([P, DC * T], BF, tag="kTt")
    for j in range(4):
        cs = slice(j * T, (j + 1) * T)
        nc.sync.dma_start(kTt[:, cs], dram["kTn"][:, cs])
    qhi_t = persist.tile([P, DC * NQ], BF, tag="qhi")
    nc.sync.dma_start(qhi_t[:], dram["qhi"])
    qlo_t = persist.tile([P, DC * NQ], BF, tag="qlo")
    nc.sync.dma_start(qlo_t[:], dram["qlo"])
    w0b = const.tile([P, _W0B_COLS], BF, tag="w0b")
    nc.sync.dma_start(w0b[:], dram["w0b"])
    vTt = persist.tile([P, DC * T], BF, tag="vTt")
    for j in range(4):
        cs = slice(j * T, (j + 1) * T)
        nc.sync.dma_start(vTt[:, cs], dram["vTn"][:, cs])

    def w0(name, r0, r1, c0, c1):
        base = _W0A[name]
        return w0a[r0:r1, base + c0:base + c1]

    Wk2s = {dc: w0("Wk2s", 0, P, dc * P, (dc + 1) * P) for dc in range(DC)}
    Wv = {dc: w0("Wv", 0, P, dc * DK, (dc + 1) * DK) for dc in range(DC)}
    Wg_hi = {dc: w0("Wg_hi", 0, P, dc * E, (dc + 1) * E) for dc in range(DC)}
    Wg_lo = {dc: w0("Wg_lo", 0, P, dc * E, (dc + 1) * E) for dc in range(DC)}
    ones_blk = w0("ones", 0, P, 0, P)
    ones_row = w0("ones", 0, 1, 0, P)        # [1,128] lhsT for bias matmul
    bq_row = w0("bq_row", 0, 1, 0, D)        # [1,512] (e d) order
    bo8 = w0("bo", 0, E, 0, D)               # [8,512] bo' = bo + bv@Wo
    Wq_f = {dc: w0b[:, dc * D:(dc + 1) * D] for dc in range(DC)}
    Wo_f = {dc: w0b[:, DC * D + dc * D:DC * D + (dc + 1) * D] for dc in range(DC)}
    qT_hi = {dc: qhi_t[:, dc * NQ:(dc + 1) * NQ] for dc in range(DC)}
    qT_lo = {dc: qlo_t[:, dc * NQ:(dc + 1) * NQ] for dc in range(DC)}

    # ---- constants ----
    ident = const.tile([P, P], FP, tag="ident")
    make_identity(nc, ident[:])
    ident_b = const.tile([P, P], BF, tag="ident_b")
    make_identity(nc, ident_b[:])
    iota8 = const.tile([P, E], FP, tag="iota8")
    nc.gpsimd.iota(iota8[:], pattern=[[1, E]], channel_multiplier=0,
                   allow_small_or_imprecise_dtypes=True)
    iota64 = const.tile([P, DK], FP, tag="iota64")
    nc.gpsimd.iota(iota64[:], pattern=[[1, DK]], channel_multiplier=0,
                   allow_small_or_imprecise_dtypes=True)

    # ---- persistent intermediates ----
    khT2 = persist.tile([P, T], BF, tag="khT2")
    vh_aug = persist.tile([P, NKC * VW], BF, tag="vh_aug")
    routerT = persist.tile([E, NQ], FP, tag="routerT")
    qselT2 = persist.tile([P, NQ], BF, tag="qselT2")
    combT = persist.tile([E, NQ], BF, tag="combT")
    attnT = [persist.tile([VW, NQ], BF, tag=f"attnT{h}", name=f"attnT{h}")
             for h in range(2)]
    g_all = persist.tile([P, 2 * NQT], FP, tag="g_all")    # gates, 2 per qt
    idx16 = persist.tile([P, P * NQT], I16, tag="idx16")   # scatter idxs per qt
    m8es = persist.tile([P, 2 * E * NQT], FP, tag="m8es")  # expert masks per qt

    # vh_aug ones columns (col 64 of each kc block)
    nc.vector.tensor_copy(
        vh_aug[:].rearrange("p (c w) -> p c w", w=VW)[:, :, DK],
        ones_blk[:, 0:NKC])

    # ---- A1: khT2 [128, T] (head-doubled kh, pre-scaled by 1/8) ----
    def emit_khT2_chunk(j):
        cs = slice(j * HD, (j + 1) * HD)
        ps = ps_m.tile([P, HD], FP, tag="ps")
        for dc in range(DC):
            nc.tensor.matmul(ps[:], Wk2s[dc][:], kTt[:, j * T + dc * HD:
                                                     j * T + (dc + 1) * HD],
                             start=(dc == 0), stop=(dc == DC - 1))
        nc.vector.tensor_copy(khT2[:, cs], ps[:])

    for j in range(4):
        emit_khT2_chunk(j)

    # ---- A2: router logits, T-layout, 3-term hi/lo (fp32-exact) ----
    for half in range(2):
        hs = slice(half * HD, (half + 1) * HD)
        ps_r = ps_m.tile([E, HD], FP, tag="ps")
        first = True
        for dc in range(DC):
            for wg, q in [(Wg_hi, qT_hi), (Wg_lo, qT_hi), (Wg_hi, qT_lo)]:
                nc.tensor.matmul(ps_r[:], wg[dc][:], q[dc][:, hs],
                                 start=first,
                                 stop=(dc == DC - 1 and q is qT_lo))
                first = False
        nc.vector.tensor_copy(routerT[:, hs], ps_r[:])

    # ---- A3 (interleaved into B blocks): vh key-major -> vh_aug ----
    def emit_vh_kc(kc):
        # shares the ps_a tag: vh use ends before phase C allocates ps_a
        ps = ps_att.tile([P, DK], FP, tag="ps_a", name="ps_vh")
        for dc in range(DC):
            nc.tensor.matmul(ps[:], vTt[:, kc * D + dc * P:kc * D + (dc + 1) * P],
                             Wv[dc][:], start=(dc == 0), stop=(dc == DC - 1))
        nc.vector.tensor_copy(vh_aug[:, kc * VW:kc * VW + DK], ps[:])

    # ---- B block for one qt: q_all, top-2 select, gather, gates ----
    def emit_B(qt):
        qs = slice(qt * P, (qt + 1) * P)
        # PE: all-expert query projection + bq bias row
        ps_qa = ps_m.tile([P, D], FP, tag="ps")
        for dc in range(DC):
            nc.tensor.matmul(ps_qa[:], qT_hi[dc][:, qs], Wq_f[dc][:],
                             start=(dc == 0), stop=False)
        nc.tensor.matmul(ps_qa[:], ones_row, bq_row, start=False, stop=True)
        qa_b = work.tile([P, D], BF, tag="qa_b")
        nc.scalar.activation(qa_b[:], ps_qa[:], AF.Copy)
        # PE: router logits transpose for this qt
        ps_lg = ps_m.tile([P, E], FP, tag="ps")
        nc.tensor.matmul(ps_lg[:], routerT[:, qs], ident[:E, :E],
                         is_transpose=True)
        lg8 = work.tile([P, E], FP, tag="lg8")
        nc.vector.tensor_copy(lg8[:], ps_lg[:])
        # top-2 selection
        m8 = work.tile([P, E], FP, tag="m8")
        nc.vector.max(out=m8[:], in_=lg8[:])
        i8 = work.tile([P, E], U32, tag="i8")
        nc.vector.max_index(i8[:], m8[:], lg8[:])
        if8 = work.tile([P, 2], FP, tag="if8")
        nc.vector.tensor_copy(if8[:], i8[:, 0:2])
        # softmax pieces: e8 + row sum, top-2 exps
        e8 = work.tile([P, E], FP, tag="e8")
        ssum = work.tile([P, 1], FP, tag="ssum")
        if ACT_ACCUM:
            nc.scalar.activation(e8[:], lg8[:], AF.Exp, accum_out=ssum[:])
        else:
            nc.scalar.activation(e8[:], lg8[:], AF.Exp)
            nc.vector.reduce_sum(ssum[:], e8[:], axis=AX.X)
        gtop = work.tile([P, 2], FP, tag="gtop")
        nc.scalar.activation(gtop[:], m8[:, 0:2], AF.Exp)
        srec = work.tile([P, 1], FP, tag="srec")
        nc.vector.reciprocal(srec[:], ssum[:])
        g = g_all[:, 2 * qt:2 * qt + 2]
        nc.vector.tensor_scalar(g, gtop[:], srec[:], None, op0=OP.mult)
        # expert one-hot masks [p, 8] per head; gather chain into qsel2
        qsel2 = work.tile([P, P], BF, tag="qsel2")
        for h in range(2):
            m8e = m8es[:, (2 * qt + h) * E:(2 * qt + h + 1) * E]
            nc.vector.tensor_scalar(m8e, iota8[:], if8[:, h:h + 1], None,
                                    op0=OP.is_equal)
            acc = qsel2[:, h * DK:(h + 1) * DK]
            nc.vector.tensor_scalar(acc, qa_b[:, 0:DK], m8e[:, 0:1], None,
                                    op0=OP.mult)
            for e in range(1, E):
                nc.vector.scalar_tensor_tensor(
                    acc, qa_b[:, e * DK:(e + 1) * DK], m8e[:, e:e + 1], acc,
                    op0=OP.mult, op1=OP.add)
        # scatter indices for phase D (int16: 64*e_h + d)
        if USE_GPSIMD_SCATTER:
            if64 = work.tile([P, 2], FP, tag="if64")
            nc.vector.tensor_scalar(if64[:], if8[:], 64.0, None, op0=OP.mult)
            for h in range(2):
                nc.vector.tensor_scalar(
                    idx16[:, qt * P + h * DK:qt * P + (h + 1) * DK],
                    iota64[:], if64[:, h:h + 1], None, op0=OP.add)
        # gate-weighted one-hot combine row (for bo' bias matmul)
        comb8 = work.tile([P, E], BF, tag="comb8")
        tmp8 = work.tile([P, E], BF, tag="tmp8")
        nc.vector.scalar_tensor_tensor(
            comb8[:], iota8[:], if8[:, 0:1],
            g[:, 0:1].to_broadcast((P, E)), op0=OP.is_equal, op1=OP.mult)
        nc.vector.scalar_tensor_tensor(
            tmp8[:], iota8[:], if8[:, 1:2],
            g[:, 1:2].to_broadcast((P, E)), op0=OP.is_equal, op1=OP.mult)
        nc.vector.tensor_tensor(comb8[:], comb8[:], tmp8[:], op=OP.add)
        # PE transposes: qsel2 -> qselT2 cols, comb8 -> combT cols
        ps_qsT = ps_m.tile([P, P], BF, tag="ps")
        nc.tensor.matmul(ps_qsT[:], qsel2[:], ident_b[:], is_transpose=True)
        nc.vector.tensor_copy(qselT2[:, qs], ps_qsT[:])
        ps_cbT = ps_m.tile([E, P], BF, tag="ps")
        nc.tensor.matmul(ps_cbT[:], comb8[:], ident_b[:], is_transpose=True)
        nc.vector.tensor_copy(combT[:, qs], ps_cbT[:])

    # ---- D1 for one qt: transpose attn back, scale, scatter to cm ----
    cms = {}
    def emit_D1(qt):
        qs = slice(qt * P, (qt + 1) * P)
        h_at = []
        for h in range(2):
            ps_at = ps_m.tile([P, VW], BF, tag="ps")
            nc.tensor.matmul(ps_at[:], attnT[h][:, qs], ident_b[:VW, :VW],
                             is_transpose=True)
            at = dpool.tile([P, VW], BF, tag=f"at{qt % 2}_{h}",
                            name=f"at{qt % 2}_{h}")
            nc.vector.tensor_copy(at[:], ps_at[:])
            h_at.append(at)
        ds = dpool.tile([P, P], BF, tag=f"ds{qt % 2}", name=f"ds{qt % 2}")
        for h in range(2):
            dinv = work.tile([P, 1], FP, tag="dinv")
            nc.vector.reciprocal(dinv[:], h_at[h][:, DK:DK + 1])
            s = work.tile([P, 1], FP, tag="s")
            nc.vector.tensor_tensor(s[:], g_all[:, 2 * qt + h:2 * qt + h + 1],
                                    dinv[:], op=OP.mult)
            nc.vector.tensor_scalar(ds[:, h * DK:(h + 1) * DK],
                                    h_at[h][:, 0:DK], s[:], None, op0=OP.mult)
        cm = dpool.tile([P, D], BF, tag=f"cm{qt % 2}", name=f"cm{qt % 2}")
        if USE_GPSIMD_SCATTER:
            nc.gpsimd.local_scatter(cm[:], ds[:],
                                    idx16[:, qt * P:(qt + 1) * P],
                                    channels=P, num_elems=D, num_idxs=P)
        else:
            for h in range(2):
                m8e = m8es[:, (2 * qt + h) * E:(2 * qt + h + 1) * E]
                for e in range(E):
                    blk = cm[:, e * DK:(e + 1) * DK]
                    if h == 0:
                        nc.vector.tensor_scalar(
                            blk, ds[:, 0:DK], m8e[:, e:e + 1], None,
                            op0=OP.mult)
                    else:
                        nc.vector.scalar_tensor_tensor(
                            blk, ds[:, DK:P], m8e[:, e:e + 1], blk,
                            op0=OP.mult, op1=OP.add)
        cms[qt] = cm

    # ---- D3 for one qt: transpose cm, output projection, store ----
    def emit_D3(qt, late):
        qs = slice(qt * P, (qt + 1) * P)
        cm = cms.pop(qt)
        cTs = []
        tp = ps_log if late else ps_m
        for ci in range(DC):
            ps_ct = tp.tile([P, P], BF, tag="ps" if tp is ps_m else "ps_lg2",
                            name="ps_ct")
            nc.tensor.matmul(ps_ct[:], cm[:, ci * P:(ci + 1) * P], ident_b[:],
                             is_transpose=True)
            cT = work.tile([P, P], BF, tag=f"cT{ci}", name=f"cT{ci}")
            nc.vector.tensor_copy(cT[:], ps_ct[:])
            cTs.append(cT)
        ps_o = ps_m.tile([P, D], FP, tag="ps")
        for ci in range(DC):
            nc.tensor.matmul(ps_o[:], cTs[ci][:], Wo_f[ci][:],
                             start=(ci == 0), stop=False)
        nc.tensor.matmul(ps_o[:], combT[:, qs], bo8, start=False, stop=True)
        o = work.tile([P, D], BF, tag="o")
        nc.scalar.activation(o[:], ps_o[:], AF.Copy)
        nc.sync.dma_start(out_d[qs, :], o[:])

    # ---- phase C for one half, with interleaved extra blocks ----
    def emit_C(half, extras):
        hs = slice(half * HD, (half + 1) * HD)
        ps_a = [ps_att.tile([VW, HD], FP, tag="ps_a", name=f"ps_a{h}")
                for h in range(2)]
        pending = None
        for kc in range(NKC):
            ps_lg2 = ps_log.tile([P, 2 * HD], FP, tag="ps_lg2")
            for h in range(2):
                rg = slice(h * DK, (h + 1) * DK)
                nc.tensor.matmul(ps_lg2[:, h * HD:(h + 1) * HD],
                                 khT2[rg, kc * P:(kc + 1) * P],
                                 qselT2[rg, hs], start=True, stop=True)
            if pending is not None:
                pkc, pex = pending
                for h in range(2):
                    nc.tensor.matmul(ps_a[h][:],
                                     vh_aug[:, pkc * VW:(pkc + 1) * VW],
                                     pex[:, h * HD:(h + 1) * HD],
                                     start=(pkc == 0), stop=(pkc == NKC - 1),
                                     skip_group_check=True)
            ex = expp.tile([P, 2 * HD], BF, tag="ex")
            nc.scalar.activation(ex[:], ps_lg2[:], AF.Exp)
            pending = (kc, ex)
            if kc in extras:
                extras[kc]()
        pkc, pex = pending
        for h in range(2):
            nc.tensor.matmul(ps_a[h][:], vh_aug[:, pkc * VW:(pkc + 1) * VW],
                             pex[:, h * HD:(h + 1) * HD],
                             start=(pkc == 0), stop=(pkc == NKC - 1),
                             skip_group_check=True)
        for h in range(2):
            nc.vector.tensor_copy(attnT[h][:, hs], ps_a[h][:])

    # ================= emission =================
    # B blocks for qts 0-3, with vh kc-groups interleaved (all 16 kc)
    for qt in range(4):
        emit_B(qt)
        for kc in range(4 * qt, 4 * qt + 4):
            emit_vh_kc(kc)

    # C half 0, with B(4..7) interleaved into the matmul stream
    emit_C(0, {1: lambda: emit_B(4), 5: lambda: emit_B(5),
               9: lambda: emit_B(6), 13: lambda: emit_B(7)})

    # C half 1, with D1(0..3) interleaved (their attnT half-0 data is ready)
    emit_C(1, {1: lambda: emit_D1(0), 5: lambda: emit_D1(1),
               9: lambda: (emit_D3(0, False), emit_D1(2)),
               13: lambda: (emit_D3(1, False), emit_D1(3))})

    # drain: D3 for 2-3, then D for qts 4-7 (ps_log pool is free now)
    emit_D3(2, True)
    emit_D1(4)
    emit_D3(3, True)
    emit_D1(5)
    emit_D3(4, True)
    emit_D1(6)
    emit_D3(5, True)
    emit_D1(7)
    emit_D3(6, True)
    emit_D3(7, True)


_PROGRAM = None


def get_program():
    global _PROGRAM
    if _PROGRAM is None:
        nc = bacc.Bacc("TRN2", target_bir_lowering=False, debug=False,
                       enable_asserts=False, num_devices=8)
        from contextlib import ExitStack
        with tile.TileContext(nc) as tc, ExitStack() as ctx:
            _emit(nc, tc, ctx)
        nc.compile()
        _PROGRAM = nc
    return _PROGRAM


def make_in_maps(query, key, value, Wg, Wk, bk, Wv, bv, Wq, bq, Wo, bo):
    import ml_dtypes
    BFNP = ml_dtypes.bfloat16

    def hilo(x):
        x = np.asarray(x, np.float32)
        hi = x.astype(BFNP)
        lo = (x - hi.astype(np.float32)).astype(BFNP)
        return hi, lo

    Wg_hi, Wg_lo = hilo(Wg)
    # kh pre-scaled by 1/8 == 1/sqrt(DK); doubled for the two head row-groups
    Wk2s = np.concatenate([np.asarray(Wk), np.asarray(Wk)], axis=1) * 0.125
    # bk shifts all logits of a query equally -> softmax-invariant: dropped.
    # (e d) ordering everywhere
    Wq_f = np.asarray(Wq).transpose(1, 0, 2).reshape(D, E * DK)
    Wo_f = np.asarray(Wo).reshape(E * DK, D)
    bq_f = np.asarray(bq).reshape(E * DK)
    # bv folds into bo since attention weights sum to 1
    bo_p = np.asarray(bo) + np.einsum('d,edm->em', np.asarray(bv, np.float32),
                                      np.asarray(Wo, np.float32))

    w0a = np.zeros((P, _W0A_COLS), BFNP)
    def put(name, rows, arr):
        base = _W0A[name]
        arr = np.asarray(arr, BFNP)
        w0a[rows, base:base + arr.shape[-1]] = arr
    for dc in range(DC):
        rows = slice(0, P)
        w0a[:, _W0A["Wk2s"] + dc * P:_W0A["Wk2s"] + (dc + 1) * P] = \
            np.asarray(Wk2s[dc * P:(dc + 1) * P, :], BFNP)
        w0a[:, _W0A["Wv"] + dc * DK:_W0A["Wv"] + (dc + 1) * DK] = \
            np.asarray(Wv, np.float32)[dc * P:(dc + 1) * P, :].astype(BFNP)
        w0a[:, _W0A["Wg_hi"] + dc * E:_W0A["Wg_hi"] + (dc + 1) * E] = \
            Wg_hi[dc * P:(dc + 1) * P, :]
        w0a[:, _W0A["Wg_lo"] + dc * E:_W0A["Wg_lo"] + (dc + 1) * E] = \
            Wg_lo[dc * P:(dc + 1) * P, :]
    w0a[:, _W0A["ones"]:_W0A["ones"] + P] = np.ones((P, P), BFNP)
    put("bq_row", 0, bq_f)
    put("bo", slice(0, E), bo_p)

    w0b = np.zeros((P, _W0B_COLS), BFNP)
    for dc in range(DC):
        w0b[:, dc * D:(dc + 1) * D] = \
            np.asarray(Wq_f, np.float32)[dc * P:(dc + 1) * P, :].astype(BFNP)
        w0b[:, DC * D + dc * D:DC * D + (dc + 1) * D] = \
            np.asarray(Wo_f, np.float32)[dc * P:(dc + 1) * P, :].astype(BFNP)

    def pack_chunks(x, n):  # [512, N] -> [128, 4*N] (dc-major columns)
        out = np.empty((P, DC * n), x.dtype)
        for dc in range(DC):
            out[:, dc * n:(dc + 1) * n] = x[dc * P:(dc + 1) * P, :]
        return np.ascontiguousarray(out)

    def pack_blocks(x, nblk, blkcols):  # [512, N] -> [128, nblk*(4*blkcols)]
        # block j = [dc0 cols | dc1 cols | dc2 cols | dc3 cols]
        out = np.empty((P, DC * nblk * blkcols), x.dtype)
        for j in range(nblk):
            for dc in range(DC):
                off = j * DC * blkcols + dc * blkcols
                out[:, off:off + blkcols] = \
                    x[dc * P:(dc + 1) * P, j * blkcols:(j + 1) * blkcols]
        return np.ascontiguousarray(out)

    shared = {"w0a": np.ascontiguousarray(w0a), "w0b": np.ascontiguousarray(w0b)}
    in_maps = []
    for b in range(4):
        kT = np.asarray(key[b], np.float32).T.astype(BFNP)     # [512, 2048]
        vT = np.asarray(value[b], np.float32).T.astype(BFNP)
        kTn = pack_blocks(kT, 4, HD)      # ncH-major blocks of 512 keys
        vTn = pack_blocks(vT, NKC, P)     # kc-major blocks of 128 keys
        for h in range(2):
            qhi, qlo = hilo(np.asarray(query[b][h * NQ:(h + 1) * NQ, :]).T)
            in_maps.append({"kTn": kTn, "vTn": vTn,
                            "qhi": pack_chunks(qhi, NQ),
                            "qlo": pack_chunks(qlo, NQ), **shared})
    return in_maps


def kernel(query, key, value, Wg, Wk, bk, Wv, bv, Wq, bq, Wo, bo):
    in_maps = make_in_maps(query, key, value, Wg, Wk, bk, Wv, bv, Wq, bq, Wo, bo)
    nc = get_program()
    res = bass_utils.run_bass_kernel_spmd(nc, in_maps, core_ids=list(range(8)))
    outs = [np.asarray(res.results[c]["out"], np.float32) for c in range(8)]
    return np.concatenate(outs, axis=0).reshape(4, T, D)
